# revision 24
# baseline (speedup 1.0000x reference)
"""Causal self-attention (B=2, T=2048, C=1024, H=16, D=64) on 8 trn2 cores.

Sharding: core c -> batch b = c // 4, head-group g = c % 4 (4 heads each).
Data-parallel over B, tensor-parallel (Megatron) over heads for the
qkv / proj linears. Each core computes its head-group's attention and a
partial output projection; the host sums the 4 partials per batch and
adds the proj bias.

v2 design (vs. the fp32r baseline):
  * bf16 operands everywhere (error gate is 2e-2; measured ~1e-4 at fp32r,
    bf16 lands ~1e-3). Halves DMA traffic and enables fast weight load.
  * Attention processed as 2 head-PAIRS. attT for the two heads of a pair
    are K=64 matmuls row-tiled into the top/bottom halves of the PE array
    (base partitions 0 / 64) writing the two banks of one [128,1024] PSUM
    tile -- they execute concurrently, and ONE exp ACTIVATE covers both
    heads (80 exp instructions instead of 160; the 352-cycle fixed cost
    per ACTIVATE was ~45% of the ACT-engine time, which is the kernel's
    critical path).
  * j (query chunk) is the outer loop so only 2 softmax-AV accumulators
    are live -> PSUM fits: pt 2x2 banks + avp 2 banks + 2 spare banks used
    to interleave the second half of the qk projection (heads 2,3) and the
    output projection INTO the ACT-bound attention window, keeping the PE
    dense and the HAM clock warm.
  * Softmax denominator from a ones-column in v_aug (row 64 of the AV
    accumulator); reciprocal on DVE on-chip; broadcast across the 64 dim
    partitions via a DRAM round trip read with partition-stride 0.
"""

import os
import sys
import types

for _p in ("/opt/trn_rl_repo", "/root/.axon_site", "/root/.axon_site/_ro/trn_rl_repo"):
    if os.path.isdir(_p) and _p not in sys.path:
        sys.path.append(_p)

import numpy as np
import ml_dtypes

import concourse.bacc as bacc
import concourse.bass as bass
import concourse.mybir as mybir
import concourse.tile as tile
from concourse.bass_utils import run_bass_kernel_spmd

# ── problem constants (hardcoded; spec.json not available at grade time) ──
B, T, C = 2, 2048, 1024
H, D = 16, 64
N_CORES = 8
HPG = 4                 # heads per group (per core)
CG = HPG * D            # 256 channels per head-group
NT = T // 512           # 4 query chunks of 512
KC = C // 128           # 8 contraction tiles for C
VW = HPG * 65           # v tile width: 4x(64 dims + ones col) = 260

F32 = mybir.dt.float32
BF16 = mybir.dt.bfloat16
EXP = mybir.ActivationFunctionType.Exp
NPBF = ml_dtypes.bfloat16

_trace_flag = [False]   # test.py can flip this to capture a profile
_last_results = [None]


def _ensure_ntff_hook():
    """Install the NTFF profile hook shim (container's antenv lacks it)."""
    if "antenv.axon_hooks" in sys.modules:
        return
    try:
        from trn_agent_boot.trn_boot import _ntff_profile_via_ctypes
    except Exception:
        return
    mod = types.ModuleType("antenv.axon_hooks")
    hook = [None]
    mod.set_axon_ntff_profile_hook = lambda h: hook.__setitem__(0, h)
    mod.get_axon_ntff_profile_hook = lambda: hook[0]
    sys.modules["antenv.axon_hooks"] = mod
    so = "/opt/axon/libaxon_pjrt.so"
    if os.path.exists(so):
        mod.set_axon_ntff_profile_hook(_ntff_profile_via_ctypes(so))


def build_nc():
    nc = bacc.Bacc("TRN2", target_bir_lowering=False, debug=False,
                   num_devices=N_CORES)

    xt_d = nc.dram_tensor("xt", [C, T], BF16, kind="ExternalInput").ap()
    wqk_d = nc.dram_tensor("wqk", [C, 2 * CG], BF16, kind="ExternalInput").ap()
    bqk_d = nc.dram_tensor("bqk", [2 * CG, 1], F32, kind="ExternalInput").ap()
    wv_d = nc.dram_tensor("wv", [C, VW], BF16, kind="ExternalInput").ap()
    bv_d = nc.dram_tensor("bv", [1, VW], BF16, kind="ExternalInput").ap()
    wp_d = nc.dram_tensor("wp", [CG, C], BF16, kind="ExternalInput").ap()
    # trineg[m, u] = -30000 where u < m else 0: DVE-added onto the diag
    # 128x128 PSUM strip before exp (exp of masked -> 0).
    trineg_d = nc.dram_tensor("trineg", [128, 128], F32, kind="ExternalInput").ap()
    ones_d = nc.dram_tensor("ones", [1, 128], BF16, kind="ExternalInput").ap()
    ones32_d = nc.dram_tensor("ones32", [1, 64], F32, kind="ExternalInput").ap()
    yt_d = nc.dram_tensor("yt", [C, T], BF16, kind="ExternalOutput").ap()
    den_d = nc.dram_tensor("den_scratch", [32, 512], F32).ap()
    rec_d = nc.dram_tensor("rec_scratch", [32, 512], F32).ap()

    with tile.TileContext(nc) as tc:
        with tc.tile_pool(name="const", bufs=1) as cp:
            # ── persistent SBUF residents ──
            xt = [cp.tile([128, T], BF16, tag=f"xt{k}", name=f"xt{k}")
                  for k in range(KC)]
            wqk = [cp.tile([128, 2 * CG], BF16, tag=f"wqk{k}", name=f"wqk{k}")
                   for k in range(KC)]
            wv = [cp.tile([128, VW], BF16, tag=f"wv{k}", name=f"wv{k}")
                  for k in range(KC)]
            bqk = [cp.tile([128, 1], F32, tag=f"bqk{m}", name=f"bqk{m}")
                   for m in range(4)]
            bv = cp.tile([1, VW], BF16, tag="bv")
            wp = [cp.tile([128, C], BF16, tag=f"wp{k}", name=f"wp{k}")
                  for k in range(2)]
            trineg = cp.tile([128, 128], F32, tag="trineg", name="trineg")
            ones = cp.tile([1, 128], BF16, tag="ones")
            ones32 = cp.tile([1, 64], F32, tag="ones32")
            # qk[0]=q heads01, qk[1]=q heads23, qk[2]=k heads01, qk[3]=k heads23
            # (head pair laid out as rows 0-63 / 64-127 of the tile)
            qk = [cp.tile([128, T], BF16, tag=f"qk{m}", name=f"qk{m}")
                  for m in range(4)]
            v_sb = [cp.tile([128, VW], BF16, tag=f"v{m}", name=f"v{m}")
                    for m in range(T // 128)]
            outT = [cp.tile([128, T], BF16, tag=f"outT{k}", name=f"outT{k}")
                    for k in range(2)]

            # stage B's first operands lead the DMA queue so the PE can
            # start the moment engine init finishes
            nc.sync.dma_start(wqk[0][:], wqk_d[0:128, :])
            nc.sync.dma_start(xt[0][:], xt_d[0:128, :])
            nc.sync.dma_start(bqk[0][:], bqk_d[0:128, :])
            for k in range(1, KC):
                nc.sync.dma_start(wqk[k][:], wqk_d[128 * k:128 * (k + 1), :])
                nc.sync.dma_start(xt[k][:], xt_d[128 * k:128 * (k + 1), :])
            for m in range(1, 4):
                nc.sync.dma_start(bqk[m][:], bqk_d[128 * m:128 * (m + 1), :])
            nc.sync.dma_start(bv[:], bv_d[:])
            for k in range(KC):
                nc.sync.dma_start(wv[k][:], wv_d[128 * k:128 * (k + 1), :])
            for k in range(2):
                nc.sync.dma_start(wp[k][:], wp_d[128 * k:128 * (k + 1), :])
            nc.sync.dma_start(trineg[:], trineg_d[:])
            nc.sync.dma_start(ones[:], ones_d[:])
            nc.sync.dma_start(ones32[:], ones32_d[:])

            # ── stage B group A: q,k for heads 0,1 (mf 0 and 2), split into
            # 4 (mf, nt-pair) groups so each group's bias-adds (DVE) overlap
            # the next group's matmuls — no PE idle bubble before stage C. ──
            with tc.tile_pool(name="psB", bufs=2, space="PSUM") as psB:
                for mf in (0, 2):
                    for ntp in (0, 1):
                        nts = (2 * ntp, 2 * ntp + 1)
                        pss = [psB.tile([128, 512], F32, tag=f"psB{q}",
                                        name=f"psB{mf}_{ntp}_{q}")
                               for q in range(2)]
                        for k in range(KC):
                            for q, nt in enumerate(nts):
                                nc.tensor.matmul(
                                    pss[q][:],
                                    wqk[k][:, 128 * mf:128 * (mf + 1)],
                                    xt[k][:, 512 * nt:512 * (nt + 1)],
                                    start=(k == 0), stop=(k == KC - 1))
                        for q, nt in enumerate(nts):
                            nc.vector.tensor_scalar_add(
                                qk[mf][:, 512 * nt:512 * (nt + 1)],
                                pss[q][:], bqk[mf][:])

            # ── stage C: v_aug [T, VW] = xt.T @ wv (+ bias & ones via K=1) ──
            with tc.tile_pool(name="psC", bufs=3, space="PSUM") as psC:
                for mt in range(T // 128):
                    ps = psC.tile([128, VW], F32, tag="psv", name=f"psv{mt}")
                    for k in range(KC):
                        nc.tensor.matmul(
                            ps[:], xt[k][:, 128 * mt:128 * (mt + 1)],
                            wv[k][:], start=(k == 0), stop=False)
                    nc.tensor.matmul(ps[:], ones[:, :], bv[:],
                                     start=False, stop=True)
                    nc.vector.tensor_copy(v_sb[mt][:], ps[:])

            # ── stage D: attention, two head-pairs. j outer, key chunk i
            # inner. attT row-tiled per pair; one exp per (pair, j, i)
            # covering both heads. Foreign PE work (stage B group B =
            # q,k heads 2,3; stage E output projection) is interleaved one
            # closure per unit to fill the ACT-bound gaps. ──
            with (
                tc.tile_pool(name="ptp", bufs=2, space="PSUM") as ptp,
                tc.tile_pool(name="avpp", bufs=1, space="PSUM") as avpp,
                tc.tile_pool(name="fxp", bufs=2, space="PSUM") as fxp,
                tc.tile_pool(name="etp", bufs=4) as etp,
                tc.tile_pool(name="rawp", bufs=2) as rawp,
                tc.tile_pool(name="recp", bufs=2) as recp,
                tc.tile_pool(name="bcp", bufs=2) as bcp,
                tc.tile_pool(name="otp", bufs=4) as otp,
            ):
                foreign = []

                # stage B group B closures: mf 1 (q23) and 3 (k23), each
                # (mf, nt) accumulated over 4 closures of 2 k-steps.
                for mf in (1, 3):
                    for nt in range(NT):
                        holder = {}
                        for kp in range(4):
                            def bgc(mf=mf, nt=nt, kp=kp, holder=holder):
                                if kp == 0:
                                    holder["ps"] = fxp.tile(
                                        [128, 512], F32, tag="fx",
                                        name=f"bg{mf}_{nt}")
                                ps = holder["ps"]
                                for k in (2 * kp, 2 * kp + 1):
                                    nc.tensor.matmul(
                                        ps[:],
                                        wqk[k][:, 128 * mf:128 * (mf + 1)],
                                        xt[k][:, 512 * nt:512 * (nt + 1)],
                                        start=(k == 0), stop=(k == KC - 1))
                                if kp == 3:
                                    nc.vector.tensor_scalar_add(
                                        qk[mf][:, 512 * nt:512 * (nt + 1)],
                                        ps[:], bqk[mf][:])
                            foreign.append(bgc)

                def mk_e(mo, nt):
                    def ecl():
                        ps = fxp.tile([128, 512], F32, tag="fx",
                                      name=f"pe{mo}_{nt}")
                        for k in range(2):
                            nc.tensor.matmul(
                                ps[:], wp[k][:, 128 * mo:128 * (mo + 1)],
                                outT[k][:, 512 * nt:512 * (nt + 1)],
                                start=(k == 0), stop=(k == 1))
                        ot = otp.tile([128, 512], BF16, tag="ot",
                                      name=f"ot{mo}_{nt}")
                        if nt == 3:
                            # tail chunk: ACT is idle by then; keep DVE free
                            nc.scalar.copy(ot[:], ps[:])
                        else:
                            nc.vector.tensor_copy(ot[:], ps[:])
                        nc.sync.dma_start(
                            yt_d[128 * mo:128 * (mo + 1),
                                 512 * nt:512 * (nt + 1)], ot[:])
                    return ecl

                for p in range(2):
                    qt = qk[p]
                    kt = qk[2 + p]
                    for j in range(NT):
                        avp0 = avpp.tile([65, 512], F32, tag="avp0",
                                         name=f"avp0_{p}_{j}")
                        avp1 = avpp.tile([65, 512], F32, tag="avp1",
                                         name=f"avp1_{p}_{j}")

                        def do_av(i, c0, et, j=j, avp0=avp0, avp1=avp1):
                            last = (i == 4 * j + 3)
                            nc.tensor.matmul(
                                avp0[:, c0:512],
                                v_sb[i][:, 130 * p:130 * p + 65],
                                et[:, c0:512], start=(i == 0), stop=last)
                            nc.tensor.matmul(
                                avp1[:, c0:512],
                                v_sb[i][:, 130 * p + 65:130 * p + 130],
                                et[:, 512:1024 - c0],
                                start=(i == 0), stop=last)

                        pending = None
                        for i in range(4 * j + 4):
                            diag = i >= 4 * j
                            c0 = 128 * (i - 4 * j) if diag else 0
                            pt = ptp.tile([128, 1024], F32, tag="pt",
                                          name=f"pt{p}_{j}_{i}")
                            # attT: both heads concurrently (row halves of
                            # the PE array). h0 lands at [c0:512], h1 at
                            # [512:1024-c0] so the merged exp range
                            # [c0:1024-c0] is contiguous with no waste.
                            nc.tensor.matmul(
                                pt[:, c0:512],
                                kt[0:64, 128 * i:128 * (i + 1)],
                                qt[0:64, 512 * j + c0:512 * (j + 1)],
                                start=True, stop=True)
                            nc.tensor.matmul(
                                pt[:, 512:1024 - c0],
                                kt[64:128, 128 * i:128 * (i + 1)],
                                qt[64:128, 512 * j + c0:512 * (j + 1)],
                                start=True, stop=True)
                            if diag:
                                # add -30000 onto the two 128x128 diag
                                # strips (DVE RMW on PSUM); exp -> 0 there
                                nc.vector.tensor_add(
                                    pt[:, c0:c0 + 128],
                                    pt[:, c0:c0 + 128], trineg[:])
                                nc.vector.tensor_add(
                                    pt[:, 512:640],
                                    pt[:, 512:640], trineg[:])
                            et = etp.tile([128, 1024], BF16, tag="et",
                                          name=f"et{p}_{j}_{i}")
                            nc.scalar.activation(et[:, c0:1024 - c0],
                                                 pt[:, c0:1024 - c0], EXP)
                            if pending is not None:
                                do_av(*pending)
                            pending = (i, c0, et)
                            if foreign:
                                foreign.pop(0)()
                        do_av(*pending)

                        # normalize both heads of the pair for this j.
                        # DVE reciprocal costs ~6.5ns/free-column no matter
                        # the partition count, so reshape the 2x512 denoms
                        # to [128,8] via a DRAM round trip (26ns recip).
                        # Raw copies come first so both avp banks free
                        # immediately for the next j's accumulation.
                        u2 = 8 * p + 2 * j
                        raws = []
                        den2 = recp.tile([128, 8], F32, tag="den2",
                                         name=f"den2_{p}_{j}")
                        for hh in range(2):
                            avp = avp0 if hh == 0 else avp1
                            raw = rawp.tile([65, 512], F32, tag=f"raw{hh}",
                                            name=f"raw{p}_{hh}_{j}")
                            nc.vector.tensor_copy(raw[:], avp[:])
                            # SBUF->SBUF DMA reshapes the 512 denominators
                            # across 64 partitions (8 per partition)
                            nc.sync.dma_start(den2[64 * hh:64 * hh + 64, :],
                                              raw[64:65, :])
                            raws.append(raw)
                        rec2 = recp.tile([128, 8], F32, tag="rec2",
                                         name=f"rec2_{p}_{j}")
                        nc.vector.reciprocal(rec2[:], den2[:])
                        if p == 1 and j == 3:
                            # final block: keep the whole chain on-chip
                            # (sb->sb un-reshape + K=1 PE broadcast) — the
                            # DRAM round trip would be fully exposed here.
                            rec_row = recp.tile([1, 1024], F32, tag="recrow",
                                                name="recrow")
                            nc.sync.dma_start(rec_row[:], rec2[:])
                            for hh in range(2):
                                bcps = fxp.tile([128, 512], F32, tag="fx",
                                                name=f"bcps{hh}")
                                nc.tensor.matmul(
                                    bcps[0:64, :], ones32[:],
                                    rec_row[0:1, 512 * hh:512 * (hh + 1)],
                                    start=True, stop=True)
                                nc.vector.tensor_mul(
                                    outT[p][64 * hh:64 * hh + 64,
                                            512 * j:512 * (j + 1)],
                                    raws[hh][0:64, :], bcps[0:64, :])
                        else:
                            nc.sync.dma_start(
                                bass.AP(rec_d.tensor, u2 * 512,
                                        [[8, 128], [1, 8]]), rec2[:])
                            for hh in range(2):
                                bc = bcp.tile([64, 512], F32, tag=f"bc{hh}",
                                              name=f"bc{p}_{hh}_{j}")
                                nc.sync.dma_start(
                                    bc[:], bass.AP(rec_d.tensor,
                                                   (u2 + hh) * 512,
                                                   [[0, 64], [1, 512]]))
                                nc.vector.tensor_mul(
                                    outT[p][64 * hh:64 * hh + 64,
                                            512 * j:512 * (j + 1)],
                                    raws[hh][0:64, :], bc[:])
                        if p == 1:
                            for mo in range(8):
                                foreign.append(mk_e(mo, j))

                # drain any remaining foreign work (E chunks for j=3)
                while foreign:
                    foreign.pop(0)()

    nc.compile()
    return nc


def _shard_inputs(x, w_qkv, b_qkv, w_proj):
    scale = 1.0 / np.sqrt(D)   # 0.125, exact power of two
    in_maps = []
    r = np.arange(128)[:, None]
    u = np.arange(128)[None, :]
    trineg = np.where(u < r, -30000.0, 0.0).astype(np.float32)
    for core in range(N_CORES):
        b, g = divmod(core, HPG)
        qs = slice(CG * g, CG * (g + 1))
        ks = slice(C + CG * g, C + CG * (g + 1))
        vs = slice(2 * C + CG * g, 2 * C + CG * (g + 1))
        wqk = np.concatenate([w_qkv[qs] * scale, w_qkv[ks]], axis=0).T
        bqk = np.concatenate([b_qkv[qs] * scale, b_qkv[ks]])[:, None]
        wv_base = w_qkv[vs].T          # [C, 256]
        wv = np.zeros((C, VW), np.float32)
        bv = np.zeros((1, VW), np.float32)
        for h in range(HPG):
            wv[:, 65 * h:65 * h + 64] = wv_base[:, 64 * h:64 * h + 64]
            bv[0, 65 * h:65 * h + 64] = b_qkv[vs][64 * h:64 * h + 64]
            bv[0, 65 * h + 64] = 1.0
        in_maps.append({
            "xt": np.ascontiguousarray(x[b].T).astype(NPBF),
            "wqk": np.ascontiguousarray(wqk).astype(NPBF),
            "bqk": np.ascontiguousarray(bqk, np.float32),
            "wv": wv.astype(NPBF),
            "bv": bv.astype(NPBF),
            "wp": np.ascontiguousarray(
                w_proj[:, CG * g:CG * (g + 1)].T).astype(NPBF),
            "trineg": trineg,
            "ones": np.ones((1, 128), NPBF),
            "ones32": np.ones((1, 64), np.float32),
        })
    return in_maps


def kernel(x, w_qkv, b_qkv, w_proj, b_proj):
    x = np.asarray(x, np.float32)
    w_qkv = np.asarray(w_qkv, np.float32)
    b_qkv = np.asarray(b_qkv, np.float32)
    w_proj = np.asarray(w_proj, np.float32)
    b_proj = np.asarray(b_proj, np.float32)

    nc = build_nc()
    in_maps = _shard_inputs(x, w_qkv, b_qkv, w_proj)
    if _trace_flag[0]:
        _ensure_ntff_hook()
    res = run_bass_kernel_spmd(nc, in_maps, core_ids=list(range(N_CORES)),
                               trace=_trace_flag[0])
    _last_results[0] = res

    y = np.empty((B, T, C), np.float32)
    for b in range(B):
        acc = np.zeros((C, T), np.float32)
        for g in range(HPG):
            acc += np.asarray(res.results[HPG * b + g]["yt"], np.float32)
        y[b] = acc.T + b_proj[None, :]
    return y


# revision 31
# speedup vs baseline: 1.1851x; 1.1851x over previous
"""Causal self-attention (B=2, T=2048, C=1024, H=16, D=64) on 8 trn2 cores.

Sharding: core c -> batch b = c // 4, head-group g = c % 4 (4 heads each).
Data-parallel over B, tensor-parallel (Megatron) over heads for the
qkv / proj linears. Each core computes its head-group's attention and a
partial output projection; the host sums the 4 partials per batch and
adds the proj bias.

Structure (v7):
  * qk / v projections run in fp8e4m3 with DoubleRow (2 K-chunks per
    matmul, 2 cols/cycle). Weights are pre-scaled x64 on the host so they
    sit in e4m3's normal range; the 1/64 descale is fused into the DVE
    bias-add. Attention itself (q.k^T logits, exp, P.v) stays bf16.
  * Attention processed as 2 head-PAIRS, query-chunk j outer. attT for the
    two heads are K=64 matmuls row-tiled into the top/bottom array halves
    (concurrent) writing one [128,1024] 2-bank PSUM tile; ONE exp ACTIVATE
    covers both heads ((N+352)-cycle ACTIVATE cost makes instruction count
    the ACT bottleneck). h1's block is left-shifted so the exp free range
    [c0:1024-c0] has no masked-waste columns.
  * Causal mask: a 128-wide -30000 strip accumulated into the diag PSUM
    block by an identity matmul before the attT matmul (exp -> 0), keeping
    the DVE out of the exp critical chain.
  * Softmax denominator from a ones-column in v_aug (row 64 of the AV
    accumulator); reciprocal must run as [128,8] (DVE reciprocal costs
    ~6.5ns per free-column regardless of partitions), reshaped via
    SBUF->SBUF DMA; broadcast across the 64 dim partitions via a DRAM
    round trip with partition-stride-0 read. The final block does the
    broadcast on-chip (K=1 matmul) since nothing would hide its latency.
  * The second half of the qk projection (heads 2,3) and the output
    projection are issued as closures interleaved one-per-unit into the
    attention loop to fill the ACT-bound PE gaps and keep the HAM clock
    warm.
"""

import os
import sys
import types

for _p in ("/opt/trn_rl_repo", "/root/.axon_site", "/root/.axon_site/_ro/trn_rl_repo"):
    if os.path.isdir(_p) and _p not in sys.path:
        sys.path.append(_p)

import numpy as np
import ml_dtypes

import concourse.bacc as bacc
import concourse.bass as bass
import concourse.mybir as mybir
import concourse.tile as tile
from concourse.bass_utils import run_bass_kernel_spmd

# ── problem constants (hardcoded; spec.json not available at grade time) ──
B, T, C = 2, 2048, 1024
H, D = 16, 64
N_CORES = 8
HPG = 4                 # heads per group (per core)
CG = HPG * D            # 256 channels per head-group
NT = T // 512           # 4 query chunks of 512
KC = C // 128           # 8 contraction tiles for C
KP = KC // 2            # 4 fp8 DoubleRow K-pair chunks
VW = HPG * 65           # v tile width: 4x(64 dims + ones col) = 260
VW8 = 272               # VW padded so the DoubleRow Ko-dim step is %16
SC = 64.0               # fp8 weight pre-scale (power of two)

F32 = mybir.dt.float32
BF16 = mybir.dt.bfloat16
F8 = mybir.dt.float8e4
EXP = mybir.ActivationFunctionType.Exp
DR = mybir.MatmulPerfMode.DoubleRow
MULT = mybir.AluOpType.mult
ADD = mybir.AluOpType.add
NPBF = ml_dtypes.bfloat16
NPF8 = ml_dtypes.float8_e4m3fn

_trace_flag = [False]   # test.py can flip this to capture a profile
_last_results = [None]


def _ensure_ntff_hook():
    """Install the NTFF profile hook shim (container's antenv lacks it)."""
    if "antenv.axon_hooks" in sys.modules:
        return
    try:
        from trn_agent_boot.trn_boot import _ntff_profile_via_ctypes
    except Exception:
        return
    mod = types.ModuleType("antenv.axon_hooks")
    hook = [None]
    mod.set_axon_ntff_profile_hook = lambda h: hook.__setitem__(0, h)
    mod.get_axon_ntff_profile_hook = lambda: hook[0]
    sys.modules["antenv.axon_hooks"] = mod
    so = "/opt/axon/libaxon_pjrt.so"
    if os.path.exists(so):
        mod.set_axon_ntff_profile_hook(_ntff_profile_via_ctypes(so))


def build_nc():
    nc = bacc.Bacc("TRN2", target_bir_lowering=False, debug=False,
                   num_devices=N_CORES)

    # fp8 operands, host-packed as [p, kp, s, f] -> [128, KP*2*f]
    xt8_d = nc.dram_tensor("xt8", [128, KP * 2 * T], F8,
                           kind="ExternalInput").ap()
    wqk8_d = nc.dram_tensor("wqk8", [128, KP * 2 * 2 * CG], F8,
                            kind="ExternalInput").ap()
    # v path stays bf16 (fp8 x/wv noise on low-entropy early tokens does
    # not average out and busts the 2e-2 gate)
    xt_d = nc.dram_tensor("xt", [C, T], BF16, kind="ExternalInput").ap()
    wv_d = nc.dram_tensor("wv", [C, VW], BF16, kind="ExternalInput").ap()
    bqk_d = nc.dram_tensor("bqk", [2 * CG, 1], F32, kind="ExternalInput").ap()
    bvf_d = nc.dram_tensor("bvf", [128, VW], BF16, kind="ExternalInput").ap()
    wp_d = nc.dram_tensor("wp", [CG, C], BF16, kind="ExternalInput").ap()
    # gg[m, w] = -30000 where w-384 < m else 0 (mask strip source)
    gg_d = nc.dram_tensor("gg", [128, 512], BF16, kind="ExternalInput").ap()
    idn_d = nc.dram_tensor("idn", [128, 128], BF16, kind="ExternalInput").ap()
    ones32_d = nc.dram_tensor("ones32", [1, 64], F32, kind="ExternalInput").ap()
    yt_d = nc.dram_tensor("yt", [C, T], BF16, kind="ExternalOutput").ap()
    rec_d = nc.dram_tensor("rec_scratch", [32, 512], F32).ap()

    with tile.TileContext(nc) as tc:
        with tc.tile_pool(name="const", bufs=1) as cp:
            # ── persistent SBUF residents ──
            xt8 = [cp.tile([128, 2, T], F8, tag=f"xt8{kp}", name=f"xt8{kp}")
                   for kp in range(KP)]
            wqk8 = [cp.tile([128, 2, 2 * CG], F8, tag=f"wqk8{kp}",
                            name=f"wqk8{kp}") for kp in range(KP)]
            xt = [cp.tile([128, T], BF16, tag=f"xt{k}", name=f"xt{k}")
                  for k in range(KC)]
            wv = [cp.tile([128, VW], BF16, tag=f"wv{k}", name=f"wv{k}")
                  for k in range(KC)]
            bqk = [cp.tile([128, 1], F32, tag=f"bqk{m}", name=f"bqk{m}")
                   for m in range(4)]
            bvf = cp.tile([128, VW], BF16, tag="bvf")
            wp = [cp.tile([128, C], BF16, tag=f"wp{k}", name=f"wp{k}")
                  for k in range(2)]
            gg = cp.tile([128, 512], BF16, tag="gg", name="gg")
            idn = cp.tile([128, 128], BF16, tag="idn", name="idn")
            ones32 = cp.tile([1, 64], F32, tag="ones32")
            # qk[0]=q heads01, qk[1]=q heads23, qk[2]=k heads01, qk[3]=k heads23
            # (head pair laid out as rows 0-63 / 64-127 of the tile)
            qk = [cp.tile([128, T], BF16, tag=f"qk{m}", name=f"qk{m}")
                  for m in range(4)]
            v_sb = [cp.tile([128, VW], BF16, tag=f"v{m}", name=f"v{m}")
                    for m in range(T // 128)]
            outT = [cp.tile([128, T], BF16, tag=f"outT{k}", name=f"outT{k}")
                    for k in range(2)]

            # stage B's first operands lead the DMA queue so the PE can
            # start the moment engine init finishes
            nc.sync.dma_start(wqk8[0][:], wqk8_d[:, 0:1024])
            nc.sync.dma_start(xt8[0][:], xt8_d[:, 0:2 * T])
            nc.sync.dma_start(bqk[0][:], bqk_d[0:128, :])
            for kp in range(1, KP):
                nc.sync.dma_start(wqk8[kp][:],
                                  wqk8_d[:, 1024 * kp:1024 * (kp + 1)])
                nc.sync.dma_start(xt8[kp][:],
                                  xt8_d[:, 2 * T * kp:2 * T * (kp + 1)])
            for m in range(1, 4):
                nc.sync.dma_start(bqk[m][:], bqk_d[128 * m:128 * (m + 1), :])
            for k in range(KC):
                nc.sync.dma_start(xt[k][:], xt_d[128 * k:128 * (k + 1), :])
                nc.sync.dma_start(wv[k][:], wv_d[128 * k:128 * (k + 1), :])
            nc.sync.dma_start(bvf[:], bvf_d[:])
            for k in range(2):
                nc.sync.dma_start(wp[k][:], wp_d[128 * k:128 * (k + 1), :])
            nc.sync.dma_start(gg[:], gg_d[:])
            nc.sync.dma_start(idn[:], idn_d[:])
            nc.sync.dma_start(ones32[:], ones32_d[:])

            # ── stage B group A: q,k for heads 0,1 (mf 0 and 2), split into
            # 4 (mf, nt-pair) groups so each group's bias-adds (DVE) overlap
            # the next group's matmuls. fp8 DoubleRow: 4 K-pair steps. ──
            with tc.tile_pool(name="psB", bufs=2, space="PSUM") as psB:
                for mf in (0, 2):
                    for ntp in (0, 1):
                        nts = (2 * ntp, 2 * ntp + 1)
                        pss = [psB.tile([128, 512], F32, tag=f"psB{q}",
                                        name=f"psB{mf}_{ntp}_{q}")
                               for q in range(2)]
                        for kp in range(KP):
                            for q, nt in enumerate(nts):
                                nc.tensor.matmul(
                                    pss[q][:],
                                    wqk8[kp][:, :, 128 * mf:128 * (mf + 1)],
                                    xt8[kp][:, :, 512 * nt:512 * (nt + 1)],
                                    start=(kp == 0), stop=(kp == KP - 1),
                                    perf_mode=DR)
                        for q, nt in enumerate(nts):
                            nc.vector.tensor_scalar(
                                qk[mf][:, 512 * nt:512 * (nt + 1)],
                                pss[q][:], 1.0 / SC, bqk[mf][:],
                                op0=MULT, op1=ADD)

            # ── stage C: v_aug [T, VW] = xt.T @ wv (fp8 DoubleRow);
            # descale + bias via one DVE scalar_tensor_tensor ──
            with tc.tile_pool(name="psC", bufs=3, space="PSUM") as psC:
                for mt in range(T // 128):
                    ps = psC.tile([128, VW], F32, tag="psv", name=f"psv{mt}")
                    for k in range(KC):
                        nc.tensor.matmul(
                            ps[:], xt[k][:, 128 * mt:128 * (mt + 1)],
                            wv[k][:], start=(k == 0), stop=(k == KC - 1))
                    nc.vector.scalar_tensor_tensor(
                        v_sb[mt][:], ps[:], 1.0, bvf[:],
                        op0=MULT, op1=ADD)

            # ── stage D: attention, two head-pairs. j outer, key chunk i
            # inner. Foreign PE work (stage B group B = q,k heads 2,3;
            # stage E output projection) interleaved one closure per unit. ──
            with (
                tc.tile_pool(name="ptp", bufs=2, space="PSUM") as ptp,
                tc.tile_pool(name="avpp", bufs=1, space="PSUM") as avpp,
                tc.tile_pool(name="fxp", bufs=2, space="PSUM") as fxp,
                tc.tile_pool(name="etp", bufs=4) as etp,
                tc.tile_pool(name="rawp", bufs=2) as rawp,
                tc.tile_pool(name="recp", bufs=2) as recp,
                tc.tile_pool(name="bcp", bufs=2) as bcp,
                tc.tile_pool(name="otp", bufs=4) as otp,
            ):
                foreign = []

                # stage B group B closures: mf 1 (q23) and 3 (k23), two
                # closures of 2 DoubleRow K-pair steps each.
                for mf in (1, 3):
                    for nt in range(NT):
                        holder = {}
                        for half in range(2):
                            def bgc(mf=mf, nt=nt, half=half, holder=holder):
                                if half == 0:
                                    holder["ps"] = fxp.tile(
                                        [128, 512], F32, tag="fx",
                                        name=f"bg{mf}_{nt}")
                                ps = holder["ps"]
                                for kp in (2 * half, 2 * half + 1):
                                    nc.tensor.matmul(
                                        ps[:],
                                        wqk8[kp][:, :,
                                                 128 * mf:128 * (mf + 1)],
                                        xt8[kp][:, :,
                                                512 * nt:512 * (nt + 1)],
                                        start=(kp == 0), stop=(kp == KP - 1),
                                        perf_mode=DR)
                                if half == 1:
                                    nc.vector.tensor_scalar(
                                        qk[mf][:, 512 * nt:512 * (nt + 1)],
                                        ps[:], 1.0 / SC, bqk[mf][:],
                                        op0=MULT, op1=ADD)
                            foreign.append(bgc)

                def mk_e(mo, nt):
                    def ecl():
                        ps = fxp.tile([128, 512], F32, tag="fx",
                                      name=f"pe{mo}_{nt}")
                        for k in range(2):
                            nc.tensor.matmul(
                                ps[:], wp[k][:, 128 * mo:128 * (mo + 1)],
                                outT[k][:, 512 * nt:512 * (nt + 1)],
                                start=(k == 0), stop=(k == 1))
                        ot = otp.tile([128, 512], BF16, tag="ot",
                                      name=f"ot{mo}_{nt}")
                        if nt == 3:
                            # tail chunk: ACT is idle by then; keep DVE free
                            nc.scalar.copy(ot[:], ps[:])
                        else:
                            nc.vector.tensor_copy(ot[:], ps[:])
                        nc.sync.dma_start(
                            yt_d[128 * mo:128 * (mo + 1),
                                 512 * nt:512 * (nt + 1)], ot[:])
                    return ecl

                for p in range(2):
                    qt = qk[p]
                    kt = qk[2 + p]
                    for j in range(NT):
                        avp0 = avpp.tile([65, 512], F32, tag="avp0",
                                         name=f"avp0_{p}_{j}")
                        avp1 = avpp.tile([65, 512], F32, tag="avp1",
                                         name=f"avp1_{p}_{j}")

                        def do_av(i, c0, et, j=j, avp0=avp0, avp1=avp1):
                            last = (i == 4 * j + 3)
                            nc.tensor.matmul(
                                avp0[:, c0:512],
                                v_sb[i][:, 130 * p:130 * p + 65],
                                et[:, c0:512], start=(i == 0), stop=last)
                            nc.tensor.matmul(
                                avp1[:, c0:512],
                                v_sb[i][:, 130 * p + 65:130 * p + 130],
                                et[:, 512:1024 - c0],
                                start=(i == 0), stop=last)

                        pending = None
                        for i in range(4 * j + 4):
                            diag = i >= 4 * j
                            c0 = 128 * (i - 4 * j) if diag else 0
                            pt = ptp.tile([128, 1024], F32, tag="pt",
                                          name=f"pt{p}_{j}_{i}")
                            # attT: both heads concurrently (row halves of
                            # the PE array). h0 lands at [c0:512], h1 at
                            # [512:1024-c0] so the merged exp range is
                            # contiguous with no masked-waste columns.
                            # Diag: a 128-col -30000 strip is accumulated
                            # first (identity @ gg); start=True clears the
                            # bank, attT(start=False) adds inside the strip
                            # and overwrites outside (per-element
                            # has_written).
                            if diag:
                                nc.tensor.matmul(
                                    pt[:, c0:c0 + 128], idn[:],
                                    gg[:, 384:512],
                                    start=True, stop=False)
                                nc.tensor.matmul(
                                    pt[:, 512:640], idn[:],
                                    gg[:, 384:512],
                                    start=True, stop=False)
                            nc.tensor.matmul(
                                pt[:, c0:512],
                                kt[0:64, 128 * i:128 * (i + 1)],
                                qt[0:64, 512 * j + c0:512 * (j + 1)],
                                start=not diag, stop=True)
                            nc.tensor.matmul(
                                pt[:, 512:1024 - c0],
                                kt[64:128, 128 * i:128 * (i + 1)],
                                qt[64:128, 512 * j + c0:512 * (j + 1)],
                                start=not diag, stop=True)
                            et = etp.tile([128, 1024], BF16, tag="et",
                                          name=f"et{p}_{j}_{i}")
                            nc.scalar.activation(et[:, c0:1024 - c0],
                                                 pt[:, c0:1024 - c0], EXP)
                            if pending is not None:
                                do_av(*pending)
                            pending = (i, c0, et)
                            if foreign:
                                foreign.pop(0)()
                        do_av(*pending)

                        # normalize both heads of the pair for this j.
                        # DVE reciprocal costs ~6.5ns/free-column no matter
                        # the partition count, so reshape the 2x512 denoms
                        # to [128,8] via SBUF->SBUF DMA (26ns recip). Raw
                        # copies come first so both avp banks free
                        # immediately for the next j's accumulation.
                        u2 = 8 * p + 2 * j
                        raws = []
                        den2 = recp.tile([128, 8], F32, tag="den2",
                                         name=f"den2_{p}_{j}")
                        for hh in range(2):
                            avp = avp0 if hh == 0 else avp1
                            raw = rawp.tile([65, 512], F32, tag=f"raw{hh}",
                                            name=f"raw{p}_{hh}_{j}")
                            nc.vector.tensor_copy(raw[:], avp[:])
                            nc.sync.dma_start(den2[64 * hh:64 * hh + 64, :],
                                              raw[64:65, :])
                            raws.append(raw)
                        rec2 = recp.tile([128, 8], F32, tag="rec2",
                                         name=f"rec2_{p}_{j}")
                        nc.vector.reciprocal(rec2[:], den2[:])
                        if p == 1 and j == 3:
                            # final block: keep the whole chain on-chip
                            # (sb->sb un-reshape + K=1 PE broadcast) — the
                            # DRAM round trip would be fully exposed here.
                            rec_row = recp.tile([1, 1024], F32, tag="recrow",
                                                name="recrow")
                            nc.sync.dma_start(rec_row[:], rec2[:])
                            for hh in range(2):
                                bcps = fxp.tile([128, 512], F32, tag="fx",
                                                name=f"bcps{hh}")
                                nc.tensor.matmul(
                                    bcps[0:64, :], ones32[:],
                                    rec_row[0:1, 512 * hh:512 * (hh + 1)],
                                    start=True, stop=True)
                                nc.vector.tensor_mul(
                                    outT[p][64 * hh:64 * hh + 64,
                                            512 * j:512 * (j + 1)],
                                    raws[hh][0:64, :], bcps[0:64, :])
                        else:
                            nc.sync.dma_start(
                                bass.AP(rec_d.tensor, u2 * 512,
                                        [[8, 128], [1, 8]]), rec2[:])
                            for hh in range(2):
                                bc = bcp.tile([64, 512], F32, tag=f"bc{hh}",
                                              name=f"bc{p}_{hh}_{j}")
                                nc.sync.dma_start(
                                    bc[:], bass.AP(rec_d.tensor,
                                                   (u2 + hh) * 512,
                                                   [[0, 64], [1, 512]]))
                                nc.vector.tensor_mul(
                                    outT[p][64 * hh:64 * hh + 64,
                                            512 * j:512 * (j + 1)],
                                    raws[hh][0:64, :], bc[:])
                        if p == 1:
                            for mo in range(8):
                                foreign.append(mk_e(mo, j))

                # drain any remaining foreign work (E chunks for j=3)
                while foreign:
                    foreign.pop(0)()

    nc.compile()
    return nc


def _pack8(a):
    """[C, F] f32 -> [128, KP*2*F] fp8 host layout [p, kp, s, f]."""
    Cd, F = a.shape
    return np.ascontiguousarray(
        a.reshape(KP, 2, 128, F).transpose(2, 0, 1, 3).reshape(128, -1)
    ).astype(NPF8)


def _shard_inputs(x, w_qkv, b_qkv, w_proj):
    scale = 1.0 / np.sqrt(D)   # 0.125, exact power of two
    in_maps = []
    r = np.arange(128)[:, None]
    w = np.arange(512)[None, :]
    gg = np.where(w - 384 < r, -30000.0, 0.0).astype(NPBF)
    idn = np.eye(128, dtype=np.float32).astype(NPBF)
    for core in range(N_CORES):
        b, g = divmod(core, HPG)
        qs = slice(CG * g, CG * (g + 1))
        ks = slice(C + CG * g, C + CG * (g + 1))
        vs = slice(2 * C + CG * g, 2 * C + CG * (g + 1))
        wqk = np.concatenate([w_qkv[qs] * scale, w_qkv[ks]], axis=0).T
        bqk = np.concatenate([b_qkv[qs] * scale, b_qkv[ks]])[:, None]
        wv_base = w_qkv[vs].T          # [C, 256]
        wv = np.zeros((C, VW), np.float32)
        bv = np.zeros((1, VW), np.float32)
        for h in range(HPG):
            wv[:, 65 * h:65 * h + 64] = wv_base[:, 64 * h:64 * h + 64]
            bv[0, 65 * h:65 * h + 64] = b_qkv[vs][64 * h:64 * h + 64]
            bv[0, 65 * h + 64] = 1.0
        in_maps.append({
            "xt8": _pack8(np.ascontiguousarray(x[b].T, np.float32)),
            "wqk8": _pack8(wqk * SC),
            "xt": np.ascontiguousarray(x[b].T).astype(NPBF),
            "wv": wv.astype(NPBF),
            "bqk": np.ascontiguousarray(bqk, np.float32),
            "bvf": np.broadcast_to(bv, (128, VW)).astype(NPBF),
            "wp": np.ascontiguousarray(
                w_proj[:, CG * g:CG * (g + 1)].T).astype(NPBF),
            "gg": gg,
            "idn": idn,
            "ones32": np.ones((1, 64), np.float32),
        })
    return in_maps


def kernel(x, w_qkv, b_qkv, w_proj, b_proj):
    x = np.asarray(x, np.float32)
    w_qkv = np.asarray(w_qkv, np.float32)
    b_qkv = np.asarray(b_qkv, np.float32)
    w_proj = np.asarray(w_proj, np.float32)
    b_proj = np.asarray(b_proj, np.float32)

    nc = build_nc()
    in_maps = _shard_inputs(x, w_qkv, b_qkv, w_proj)
    if _trace_flag[0]:
        _ensure_ntff_hook()
    res = run_bass_kernel_spmd(nc, in_maps, core_ids=list(range(N_CORES)),
                               trace=_trace_flag[0])
    _last_results[0] = res

    y = np.empty((B, T, C), np.float32)
    for b in range(B):
        acc = np.zeros((C, T), np.float32)
        for g in range(HPG):
            acc += np.asarray(res.results[HPG * b + g]["yt"], np.float32)
        y[b] = acc.T + b_proj[None, :]
    return y
